# revision 1
# baseline (speedup 1.0000x reference)
"""MeshFC kernel for 8x TRN2 NeuronCores.

Computes: out = inputs @ w + biases, where
  w[i,o] = ||in_pos[i]-out_pos[o]|| - ||init_in_pos[i]-init_out_pos[o]||

Sharding: tensor-parallel on the output dim (8 x 1024 columns). Each core:
  - generates its weight column block on-chip via the PE using the
    augmented-inner-product identity dist^2 = ||a||^2 - 2 a.b + ||b||^2
    (a single K=7 fp32 matmul per tile), sqrt on ScalarE, subtract on DVE
  - runs the main [4096,2048]x[2048,1024] matmul in float32r (FP22)
Host side: pre-transposes/pre-tiles inputs so every DMA is contiguous,
and concatenates the 8 per-core [4096,1024] outputs.
"""

import os
from contextlib import ExitStack

import numpy as np

NUM_IN, NUM_OUT, SD, BATCH = 2048, 8192, 5, 4096
N_CORES = 8
O_SHARD = NUM_OUT // N_CORES  # 1024
B_TILES = BATCH // 128  # 32
K_TILES = NUM_IN // 128  # 16
O_HALves = O_SHARD // 512  # 2

_CACHE = {}


def _build_bass(variant=""):
    import concourse.bass as bass  # noqa: F401
    import concourse.mybir as mybir
    from concourse import bacc
    from concourse.tile import TileContext

    fp32 = mybir.dt.float32
    fp32r = mybir.dt.float32r
    fp16 = mybir.dt.float16

    # main-matmul dtype: fp16 runs at 1 cycle/row (fp32r: 2, fp32: 4+) with
    # accuracy on par with fp22 (10-bit rounded vs 13-bit truncated mantissa)
    mmdt = fp32r if "fp32r" in variant else fp16

    # Bacc (not plain Bass): its compile() runs generate_event_semaphores +
    # move_matmul_waits_to_ldweights, which split multi-waits that exceed the
    # per-instruction HW sync-wait budget.
    nc = bacc.Bacc("TRN2", name="meshfc")

    xT = nc.dram_tensor("xT", [B_TILES, 128, NUM_IN], mmdt, kind="ExternalInput")
    # packed [aC | aI | bC | bI] along the free axis -> single DMA, single wait
    AB_W = 2 * NUM_IN + 2 * O_SHARD
    ab = nc.dram_tensor("ab", [7, AB_W], fp32, kind="ExternalInput")
    # [bias | ones(128)] packed on one partition
    bias = nc.dram_tensor("bias", [1, O_SHARD + 128], mmdt, kind="ExternalInput")
    out = nc.dram_tensor("out", [BATCH, O_SHARD], fp32, kind="ExternalOutput")

    with ExitStack() as ctx:
        tc = ctx.enter_context(TileContext(nc))
        const = ctx.enter_context(tc.tile_pool(name="const", bufs=1))
        wps = mps = tmp = xpool = opool = None
        if "nowgen" not in variant:
            wps = ctx.enter_context(tc.tile_pool(name="wps", bufs=2, space="PSUM"))
            tmp = ctx.enter_context(tc.tile_pool(name="tmp", bufs=2))
        if "nomm" not in variant:
            mps = ctx.enter_context(tc.tile_pool(name="mps", bufs=2, space="PSUM"))
            xpool = ctx.enter_context(tc.tile_pool(name="xp", bufs=3))
            opool = ctx.enter_context(tc.tile_pool(name="op", bufs=3))

        # --- constants ---
        ab_sb = const.tile([7, AB_W], fp32, name="ab_sb")
        nc.sync.dma_start(out=ab_sb, in_=ab[:, :])
        aC_sb = ab_sb[:, 0:NUM_IN]
        aI_sb = ab_sb[:, NUM_IN : 2 * NUM_IN]
        bC_sb = ab_sb[:, 2 * NUM_IN : 2 * NUM_IN + O_SHARD]
        bI_sb = ab_sb[:, 2 * NUM_IN + O_SHARD : AB_W]

        # bias lives on one partition; it is added into PSUM via a K=1 matmul
        biasones_sb = const.tile([1, O_SHARD + 128], mmdt, name="biasones_sb")
        nc.sync.dma_start(out=biasones_sb, in_=bias[:, :])
        bias_sb = biasones_sb[:, 0:O_SHARD]
        ones_sb = biasones_sb[:, O_SHARD : O_SHARD + 128]

        # resident weight block: [128, K_TILES, O_SHARD] = 8 MB
        # float32r so the DVE write rounds to FP22 for the fp32r matmul
        w_sb = const.tile([128, K_TILES, O_SHARD], mmdt, name="w_sb")

        # optional on-device repetition for slope timing (variant "repN")
        n_rep = 1
        for tok in variant.split(","):
            if tok.startswith("rep"):
                n_rep = int(tok[3:])

        # --- weight generation ---
        for _rep in range(n_rep):
            _build_body(nc, tc, variant, const, wps, mps, tmp, xpool, opool,
                        aC_sb, aI_sb, bC_sb, bI_sb, bias_sb, ones_sb, w_sb,
                        xT, out, fp32, mmdt)

    nc.finalize()
    return nc


def _build_body(nc, tc, variant, const, wps, mps, tmp, xpool, opool,
                aC_sb, aI_sb, bC_sb, bI_sb, bias_sb, ones_sb, w_sb,
                xT, out, fp32, mmdt):
    import concourse.mybir as mybir  # noqa: F401

    if True:
        if "nowgen" not in variant:
            for kt in range(K_TILES):
                for oh in range(O_HALves):
                    osl = slice(oh * 512, (oh + 1) * 512)
                    psC = wps.tile([128, 512], fp32, tag="psC", bufs=2)
                    psI = wps.tile([128, 512], fp32, tag="psI", bufs=2)
                    nc.tensor.matmul(
                        psC,
                        aC_sb[:, kt * 128 : (kt + 1) * 128],
                        bC_sb[:, osl],
                        start=True,
                        stop=True,
                    )
                    nc.tensor.matmul(
                        psI,
                        aI_sb[:, kt * 128 : (kt + 1) * 128],
                        bI_sb[:, osl],
                        start=True,
                        stop=True,
                    )
                    # clamp dist^2 to >=0 on DVE (HW fp32 rounding can push
                    # the closest pair slightly negative -> sqrt NaN), then
                    # sqrt in place in SBUF. In-place PSUM activation crashes
                    # the exec unit, so everything lands in SBUF tmps.
                    sC = tmp.tile([128, 512], fp32, tag="sC", bufs=2)
                    sI = tmp.tile([128, 512], fp32, tag="sI", bufs=2)
                    nc.vector.tensor_scalar_max(sC, psC, 0.0)
                    nc.vector.tensor_scalar_max(sI, psI, 0.0)
                    nc.scalar.sqrt(sC, sC)
                    nc.scalar.sqrt(sI, sI)
                    nc.vector.tensor_sub(w_sb[:, kt, osl], sC, sI)

        # --- main matmul: out[b,o] = sum_k x[b,k] w[k,o] (+bias) ---
        if "nomm" in variant:
            return
        for bt in range(B_TILES):
            xt = xpool.tile([128, NUM_IN], mmdt, name="xt")
            if "nodma" not in variant:
                nc.sync.dma_start(out=xt, in_=xT[bt])
            ot = opool.tile([128, O_SHARD], fp32, name="ot")
            # pre-touch: absorbs the out-DMA slot-release wait on ScalarE so
            # the real drains below stay within the HW sync-wait slot limit
            if "nodrain" not in variant:
                nc.scalar.mul(ot[0:1, 0:1], ot[0:1, 0:1], 0.0)
            for oh in range(O_HALves):
                osl = slice(oh * 512, (oh + 1) * 512)
                ps = mps.tile([128, 512], fp32, tag="ps", bufs=2)
                for kt in range(K_TILES):
                    nc.tensor.matmul(
                        ps,
                        xt[:, kt * 128 : (kt + 1) * 128],
                        w_sb[:, kt, osl],
                        start=(kt == 0),
                        stop=("nobias" in variant and kt == K_TILES - 1),
                    )
                # += bias (broadcast over rows via rank-1 matmul)
                if "nobias" not in variant:
                    nc.tensor.matmul(
                        ps, ones_sb[:, :], bias_sb[:, osl], start=False, stop=True
                    )
                if "nodrain" not in variant:
                    nc.scalar.copy(ot[:, osl], ps)
            if "nodrain" not in variant:
                nc.sync.dma_start(out=out[bt * 128 : (bt + 1) * 128, :], in_=ot)


def _prep_inputs(inputs, init_in_pos, init_out_pos, in_pos, out_pos, biases,
                 mm_np_dt=np.float16):
    x = np.ascontiguousarray(np.asarray(inputs, dtype=np.float32))
    a = np.asarray(in_pos, dtype=np.float32).reshape(NUM_IN, SD)
    a0 = np.asarray(init_in_pos, dtype=np.float32).reshape(NUM_IN, SD)
    b = np.asarray(out_pos, dtype=np.float32).reshape(NUM_OUT, SD)
    b0 = np.asarray(init_out_pos, dtype=np.float32).reshape(NUM_OUT, SD)
    bias = np.asarray(biases, dtype=np.float32).reshape(NUM_OUT)

    # [bt, p, kt*128+b'] = x[bt*128+b', kt*128+p]
    xT = np.ascontiguousarray(
        x.reshape(B_TILES, 128, K_TILES, 128).transpose(0, 3, 2, 1).astype(mm_np_dt)
    ).reshape(B_TILES, 128, NUM_IN)

    def aug_a(p):
        return np.concatenate(
            [p.T, (p * p).sum(1)[None, :], np.ones((1, p.shape[0]), np.float32)], 0
        ).astype(np.float32)

    def aug_b(q):
        return np.concatenate(
            [-2.0 * q.T, np.ones((1, q.shape[0]), np.float32), (q * q).sum(1)[None, :]],
            0,
        ).astype(np.float32)

    aCv, aIv = aug_a(a), aug_a(a0)
    bC_full, bI_full = aug_b(b), aug_b(b0)

    in_maps = []
    for c in range(N_CORES):
        sl = slice(c * O_SHARD, (c + 1) * O_SHARD)
        ab = np.ascontiguousarray(
            np.concatenate([aCv, aIv, bC_full[:, sl], bI_full[:, sl]], axis=1)
        )
        in_maps.append(
            {
                "xT": xT,
                "ab": ab,
                "bias": np.ascontiguousarray(
                    np.concatenate([bias[sl], np.ones(128, np.float32)]).astype(
                        mm_np_dt
                    )
                )[None, :],
            }
        )
    return in_maps


def _run(in_maps, trace=False):
    from concourse.bass_utils import run_bass_kernel_spmd

    if "nc" not in _CACHE:
        _CACHE["nc"] = _build_bass()
    nc = _CACHE["nc"]
    res = run_bass_kernel_spmd(
        nc, in_maps, core_ids=list(range(N_CORES)), trace=trace
    )
    outs = [r["out"] for r in res.results]
    return np.concatenate(outs, axis=1), res


def kernel(**inputs) -> np.ndarray:
    in_maps = _prep_inputs(**inputs)
    out, _ = _run(in_maps, trace=bool(os.environ.get("MESHFC_TRACE")))
    return out



# revision 2
# speedup vs baseline: 1.3215x; 1.3215x over previous
"""MeshFC kernel for 8x TRN2 NeuronCores.

Computes: out = inputs @ w + biases, where
  w[i,o] = ||in_pos[i]-out_pos[o]|| - ||init_in_pos[i]-init_out_pos[o]||

Sharding: tensor-parallel on the output dim (8 x 1024 columns). Each core:
  - generates its weight column block on-chip via the PE using the
    augmented-inner-product identity dist^2 = ||a||^2 - 2 a.b + ||b||^2.
    The fp32 inner products are emulated with a bf16 hi/lo split
    (a = ah+al, b = bh+bl; a.b ~ ah.bh + ah.bl + al.bh, error ~2^-18)
    so the wg matmul is a single-pass bf16 stream (1 row/cycle) instead
    of a 2-pass fp32r stream that keeps the PE ~50% idle and the HAM
    clock-gate stuck at 1.2 GHz.
  - runs the main [4096,2048]x[2048,1024] matmul in fp16
  - biases are added on the Vector engine during the PSUM drain (a
    host-replicated [128,1024] broadcast tile), not via PE rank-1 matmuls
  - a burst of dummy warm-up matmuls at t=0 releases the HAM clock-gate
    (cold PE runs at 1.2 GHz; ~3.4us of sustained activity -> 2.4 GHz)
    while the input DMAs are still in flight.
Host side: pre-transposes/pre-tiles inputs so every DMA is contiguous,
and concatenates the 8 per-core [4096,1024] outputs.
"""

import os
from contextlib import ExitStack

import numpy as np

NUM_IN, NUM_OUT, SD, BATCH = 2048, 8192, 5, 4096
N_CORES = 8
O_SHARD = NUM_OUT // N_CORES  # 1024
B_TILES = BATCH // 128  # 32
K_TILES = NUM_IN // 128  # 16
O_HALves = O_SHARD // 512  # 2
KAUG = 3 * SD + 4  # 19: [ah ah al | na_h na_l 1 1] . [-2bh -2bl -2bh | 1 1 nb_h nb_l]
N_WARM = 36

_CACHE = {}


def _build_bass(variant=""):
    import concourse.bass as bass  # noqa: F401
    import concourse.mybir as mybir
    from concourse import bacc
    from concourse.tile import TileContext

    fp32 = mybir.dt.float32
    bf16 = mybir.dt.bfloat16
    fp16 = mybir.dt.float16

    mmdt = fp16

    # Bacc (not plain Bass): its compile() runs generate_event_semaphores +
    # move_matmul_waits_to_ldweights, which split multi-waits that exceed the
    # per-instruction HW sync-wait budget.
    nc = bacc.Bacc("TRN2", name="meshfc")

    xT = nc.dram_tensor("xT", [B_TILES, 128, NUM_IN], mmdt, kind="ExternalInput")
    # packed [A_C | A_I | B_C | B_I] along the free axis -> single DMA
    AB_W = 2 * NUM_IN + 2 * O_SHARD
    ab = nc.dram_tensor("ab", [KAUG, AB_W], bf16, kind="ExternalInput")
    # bias replicated to 128 partitions host-side; added on DVE during drain
    bias = nc.dram_tensor("bias", [128, O_SHARD], fp32, kind="ExternalInput")
    out = nc.dram_tensor("out", [BATCH, O_SHARD], fp32, kind="ExternalOutput")

    with ExitStack() as ctx:
        tc = ctx.enter_context(TileContext(nc))
        const = ctx.enter_context(tc.tile_pool(name="const", bufs=1))
        wps = ctx.enter_context(tc.tile_pool(name="wps", bufs=2, space="PSUM"))
        tmp = ctx.enter_context(tc.tile_pool(name="tmp", bufs=2))
        mps = ctx.enter_context(tc.tile_pool(name="mps", bufs=2, space="PSUM"))
        xpool = ctx.enter_context(tc.tile_pool(name="xp", bufs=3))
        opool = ctx.enter_context(tc.tile_pool(name="op", bufs=3))
        warmps = ctx.enter_context(tc.tile_pool(name="wmps", bufs=2, space="PSUM"))

        # --- PE warm-up: release the HAM clock-gate during input DMA ---
        warm_sb = const.tile([128, 512], bf16, name="warm_sb")
        nc.gpsimd.memset(warm_sb, 0.0)
        for _ in range(N_WARM):
            wp = warmps.tile([128, 512], fp32, tag="wm", bufs=2)
            nc.tensor.matmul(wp, warm_sb[:, 0:128], warm_sb, start=True, stop=True)

        # --- constants ---
        ab_sb = const.tile([KAUG, AB_W], bf16, name="ab_sb")
        nc.sync.dma_start(out=ab_sb, in_=ab[:, :])
        aC_sb = ab_sb[:, 0:NUM_IN]
        aI_sb = ab_sb[:, NUM_IN : 2 * NUM_IN]
        bC_sb = ab_sb[:, 2 * NUM_IN : 2 * NUM_IN + O_SHARD]
        bI_sb = ab_sb[:, 2 * NUM_IN + O_SHARD : AB_W]

        bias_sb = const.tile([128, O_SHARD], fp32, name="bias_sb")
        nc.sync.dma_start(out=bias_sb, in_=bias[:, :])

        # resident weight block: [128, K_TILES, O_SHARD] = 4 MB fp16
        w_sb = const.tile([128, K_TILES, O_SHARD], mmdt, name="w_sb")

        # --- weight generation (oh outer so main MMs can start early) ---
        for oh in range(O_HALves):
            osl = slice(oh * 512, (oh + 1) * 512)
            for kt in range(K_TILES):
                ksl = slice(kt * 128, (kt + 1) * 128)
                psC = wps.tile([128, 512], fp32, tag="psC", bufs=2)
                psI = wps.tile([128, 512], fp32, tag="psI", bufs=2)
                nc.tensor.matmul(psC, aC_sb[:, ksl], bC_sb[:, osl], start=True, stop=True)
                nc.tensor.matmul(psI, aI_sb[:, ksl], bI_sb[:, osl], start=True, stop=True)
                # clamp dist^2 to >=0 on DVE (fp32 rounding can push the
                # closest pair slightly negative -> sqrt NaN), then sqrt in
                # SBUF. In-place PSUM activation crashes the exec unit.
                sC = tmp.tile([128, 512], fp32, tag="sC", bufs=2)
                sI = tmp.tile([128, 512], fp32, tag="sI", bufs=2)
                nc.vector.tensor_scalar_max(sC, psC, 0.0)
                nc.vector.tensor_scalar_max(sI, psI, 0.0)
                nc.scalar.sqrt(sC, sC)
                nc.scalar.sqrt(sI, sI)
                nc.vector.tensor_sub(w_sb[:, kt, osl], sC, sI)

        # --- main matmul: out[b,o] = sum_k x[b,k] w[k,o] + bias[o] ---
        for bt in range(B_TILES):
            xt = xpool.tile([128, NUM_IN], mmdt, name="xt")
            nc.sync.dma_start(out=xt, in_=xT[bt])
            ot = opool.tile([128, O_SHARD], fp32, name="ot")
            # pre-touch: absorbs the out-DMA slot-release wait on ScalarE so
            # the real drains below stay within the HW sync-wait slot limit
            nc.scalar.mul(ot[0:1, 0:1], ot[0:1, 0:1], 0.0)
            for oh in range(O_HALves):
                osl = slice(oh * 512, (oh + 1) * 512)
                ps = mps.tile([128, 512], fp32, tag="ps", bufs=2)
                for kt in range(K_TILES):
                    nc.tensor.matmul(
                        ps,
                        xt[:, kt * 128 : (kt + 1) * 128],
                        w_sb[:, kt, osl],
                        start=(kt == 0),
                        stop=(kt == K_TILES - 1),
                    )
                # drain with bias add on DVE (was: rank-1 PE matmul + scalar copy)
                nc.vector.tensor_add(ot[:, osl], ps, bias_sb[:, osl])
            nc.sync.dma_start(out=out[bt * 128 : (bt + 1) * 128, :], in_=ot)

    nc.finalize()
    return nc


def _hi_lo(v):
    """Split fp32 array into bf16 hi + bf16 lo with v ~ hi+lo to ~2^-17."""
    import ml_dtypes

    bf = ml_dtypes.bfloat16
    hi = v.astype(bf)
    lo = (v - hi.astype(np.float32)).astype(bf)
    return hi, lo


def _aug_pair(p, q):
    """Augmented bf16 row blocks for a (in, [N,5]) and b (out, [M,5]) such
    that A.T @ B ~= ||a||^2 - 2 a.b + ||b||^2 in one bf16 matmul pass.

    A rows: [ah(5); ah(5); al(5); na_h; na_l; 1; 1]        -> [19, N]
    B rows: [-2bh(5); -2bl(5); -2bh(5); 1; 1; nb_h; nb_l]  -> [19, M]
    """
    import ml_dtypes

    bf = ml_dtypes.bfloat16
    n, m = p.shape[0], q.shape[0]
    na = (p.astype(np.float64) ** 2).sum(1).astype(np.float32)
    nb = (q.astype(np.float64) ** 2).sum(1).astype(np.float32)
    ah, al = _hi_lo(p)
    bh, bl = _hi_lo(q)
    nah, nal = _hi_lo(na)
    nbh, nbl = _hi_lo(nb)
    ones_n = np.ones((1, n), bf)
    ones_m = np.ones((1, m), bf)
    A = np.concatenate(
        [ah.T, ah.T, al.T, nah[None, :], nal[None, :], ones_n, ones_n], 0
    ).astype(bf)
    B = np.concatenate(
        [-2 * bh.T, -2 * bl.T, -2 * bh.T, ones_m, ones_m, nbh[None, :], nbl[None, :]], 0
    ).astype(bf)
    assert A.shape == (KAUG, n) and B.shape == (KAUG, m)
    return A, B


def _prep_inputs(inputs, init_in_pos, init_out_pos, in_pos, out_pos, biases,
                 mm_np_dt=np.float16):
    x = np.ascontiguousarray(np.asarray(inputs, dtype=np.float32))
    a = np.asarray(in_pos, dtype=np.float32).reshape(NUM_IN, SD)
    a0 = np.asarray(init_in_pos, dtype=np.float32).reshape(NUM_IN, SD)
    b = np.asarray(out_pos, dtype=np.float32).reshape(NUM_OUT, SD)
    b0 = np.asarray(init_out_pos, dtype=np.float32).reshape(NUM_OUT, SD)
    bias = np.asarray(biases, dtype=np.float32).reshape(NUM_OUT)

    # [bt, p, kt*128+b'] = x[bt*128+b', kt*128+p]
    xT = np.ascontiguousarray(
        x.reshape(B_TILES, 128, K_TILES, 128).transpose(0, 3, 2, 1).astype(mm_np_dt)
    ).reshape(B_TILES, 128, NUM_IN)

    A_C, B_C_full = _aug_pair(a, b)
    A_I, B_I_full = _aug_pair(a0, b0)

    in_maps = []
    for c in range(N_CORES):
        sl = slice(c * O_SHARD, (c + 1) * O_SHARD)
        ab = np.ascontiguousarray(
            np.concatenate([A_C, A_I, B_C_full[:, sl], B_I_full[:, sl]], axis=1)
        )
        bias_bc = np.ascontiguousarray(
            np.broadcast_to(bias[sl][None, :], (128, O_SHARD)).astype(np.float32)
        )
        in_maps.append({"xT": xT, "ab": ab, "bias": bias_bc})
    return in_maps


def _run(in_maps, trace=False):
    from concourse.bass_utils import run_bass_kernel_spmd

    if "nc" not in _CACHE:
        _CACHE["nc"] = _build_bass()
    nc = _CACHE["nc"]
    res = run_bass_kernel_spmd(
        nc, in_maps, core_ids=list(range(N_CORES)), trace=trace
    )
    outs = [r["out"] for r in res.results]
    return np.concatenate(outs, axis=1), res


def kernel(**inputs) -> np.ndarray:
    in_maps = _prep_inputs(**inputs)
    out, _ = _run(in_maps, trace=bool(os.environ.get("MESHFC_TRACE")))
    return out
